# revision 1
# baseline (speedup 1.0000x reference)
"""NPMLPHead sampling kernel for Trainium2 (Bass/Tile), SPMD over 8 cores.

Strategy (data-parallel over batch, per sharding hint):
  - B=16 images -> 2 per core; full (tiny) MLP weights replicated per core.
  - L0 (128 of 16K positions, C=256): 4-byte-element strided DMA gathers,
    one instruction per patch ([c_part, (b ch)]), in SORTED index order
    with contiguous sorted blocks per sequencer (DRAM row-buffer locality);
    split across the SP/Act HWDGE rings and the Pool SWDGE ring. The host
    un-permutes the patch axis afterwards.
  - L1 (128 of 4K, C=512): instead of 131K 4-byte descriptors, gather the
    ~55 DISTINCT h-rows (256B contiguous w-runs, one DMA instruction per
    (image, h) over all channels) into an SBUF stash, then extract each
    patch's w-column with per-patch [128, ch] DVE/Pool copies. This halves
    the 4B descriptor load, which is the empirical bottleneck (~1.7
    descriptors/ns system-wide regardless of issue schedule). The stash is
    reused between images (image-1 rows wait on image-0 extraction).
  - L2 (128 of 1K, C=1024): stream the whole shard early with big
    bandwidth-efficient descriptors, compute the MLP on ALL positions in
    the native [C, HW] layout (contraction on partitions), select patches
    at the end with a one-hot PE matmul (q lands on partitions via the
    layer-2 stationary operand).
  - Engine roles: gather DMAs are completion-throttled, so Scalar/Act keeps
    its compute chain (relu/norm/store) after only a bounded gather block;
    Sync and Pool carry the bulk of the DMA instructions.
  - All matmuls float32r (single-pass fp32, ~tf32 rounding); norm =
    Square+accum -> sqrt -> reciprocal -> mul (eps dropped; |y| >> eps).
"""

import sys

sys.path.insert(0, "/opt/trn_rl_repo")

import numpy as np

B = 16
N_CORES = 8
B_LOC = B // N_CORES  # 2
P = 128  # NUM_PATCHES
NCD = 256  # MLP width
LEVELS = [(256, 128), (512, 64), (1024, 32)]  # (C, H) per level
EPS = 1e-7

# L0 single-patch gathers: contiguous sorted blocks per engine.
# L1 uses 256-byte row-gathers (one instr per distinct h per image) into an
# SBUF stash, then per-patch DVE extraction; rows split across sync/pool.
GATHER_BLOCKS = [("sync", 50), ("gpsimd", 43), ("scalar", 35)]
L1_ROW_SYNC_FRAC = 0.6


def _build(idx_vals):
    """Build the per-core Bass kernel. idx_vals: 3 int arrays of 128 patch
    ids. L0/L1 are gathered in sorted-index order (host un-permutes)."""
    import concourse.bass as bass
    import concourse.tile as tile
    from concourse import bacc, mybir

    f32 = mybir.dt.float32
    fr = mybir.dt.float32r
    AF = mybir.ActivationFunctionType

    nc = bacc.Bacc(None)

    feats, w1s, b1s, w2s, b2s = [], [], [], [], []
    for l, (C, H) in enumerate(LEVELS):
        feats.append(
            nc.dram_tensor(f"feat{l}", [B_LOC, C, H, H], fr, kind="ExternalInput")
        )
        w1s.append(nc.dram_tensor(f"w1_{l}", [C, NCD], fr, kind="ExternalInput"))
        b1s.append(nc.dram_tensor(f"b1_{l}", [NCD], fr, kind="ExternalInput"))
        w2s.append(nc.dram_tensor(f"w2_{l}", [NCD, NCD], fr, kind="ExternalInput"))
        b2s.append(nc.dram_tensor(f"b2_{l}", [NCD], fr, kind="ExternalInput"))
    # one-hot select for level 2 (original patch order)
    oh = nc.dram_tensor("oh2", [8, 128, P], fr, kind="ExternalInput")
    out = nc.dram_tensor("out", [3, B_LOC, P, NCD], f32, kind="ExternalOutput")

    C2, H2 = LEVELS[2]
    HW2 = H2 * H2  # 1024
    NCH2 = C2 // 128  # 8
    QC2 = HW2 // 128  # 8 q-chunks

    sv = {l: np.sort(np.asarray(idx_vals[l]).astype(np.int64)) for l in (0, 1)}

    with tile.TileContext(nc) as tc:
        with (
            tc.tile_pool(name="consts", bufs=1) as consts,
            tc.tile_pool(name="xt", bufs=1) as xtp,
            tc.tile_pool(name="l1s", bufs=1) as l1pool,
            tc.tile_pool(name="work", bufs=4) as work,
            tc.tile_pool(name="psum", bufs=2, space=bass.MemorySpace.PSUM) as psum,
            tc.tile_pool(name="psum1", bufs=1, space=bass.MemorySpace.PSUM) as psum1,
        ):
            ones_f = consts.tile([1, 512], f32, tag="ones_f")
            nc.vector.memset(ones_f[:], 1.0)
            ones = consts.tile([1, 512], fr, tag="ones")
            nc.scalar.copy(ones[:], ones_f[:])

            # --- t2 streams + L2 weights + oh first on scalar ---
            t2s = []
            for b in range(B_LOC):
                t2 = xtp.tile([128, NCH2 * HW2], fr, tag=f"t2_{b}", name=f"t2_{b}")
                nc.scalar.dma_start(
                    t2[:].rearrange("cp (cc hw) -> cp cc hw", hw=HW2),
                    feats[2][b].rearrange("(cc cp) h w -> cp cc (h w)", cp=128),
                )
                t2s.append(t2)

            w1_sb, w2_sb, b1_sb, b2_sb, xts = {}, {}, {}, {}, {}

            def load_weights(l, eng):
                C, H = LEVELS[l]
                n_ch = C // 128
                t = consts.tile([128, n_ch * NCD], fr, tag=f"w1_{l}", name=f"w1sb{l}")
                eng.dma_start(
                    t[:].rearrange("cp (ch n) -> cp ch n", n=NCD),
                    w1s[l][:].rearrange("(ch cp) n -> cp ch n", cp=128),
                )
                w1_sb[l] = t
                t = consts.tile([128, 2 * NCD], fr, tag=f"w2_{l}", name=f"w2sb{l}")
                eng.dma_start(
                    t[:].rearrange("cp (ch n) -> cp ch n", n=NCD),
                    w2s[l][:].rearrange("(ch cp) n -> cp ch n", cp=128),
                )
                w2_sb[l] = t
                t = consts.tile([1, NCD], fr, tag=f"b1_{l}", name=f"b1sb{l}")
                eng.dma_start(t[:], b1s[l][:].rearrange("(o n) -> o n", o=1))
                b1_sb[l] = t
                t = consts.tile([1, NCD], fr, tag=f"b2_{l}", name=f"b2sb{l}")
                eng.dma_start(t[:], b2s[l][:].rearrange("(o n) -> o n", o=1))
                b2_sb[l] = t

            load_weights(2, nc.scalar)
            oh_sb = consts.tile([128, QC2 * P], fr, tag="oh2")
            nc.scalar.dma_start(
                oh_sb[:].rearrange("ql (qc p) -> ql qc p", p=P),
                oh[:].rearrange("qc ql p -> ql qc p"),
            )

            for l in (0, 1):
                C, H = LEVELS[l]
                n_ch = C // 128
                xts[l] = xtp.tile(
                    [128, B_LOC * n_ch * P], fr, tag=f"xt_{l}", name=f"xt{l}"
                )

            # --- L1 row-gathers: distinct sorted h values, 256B w-rows,
            # stash split into two h-halves (A/B) so image-1 rows pipeline
            # behind image-0's per-half extraction ---
            H1 = LEVELS[1][1]  # 64
            hvals = sorted(set(int(q) >> 6 for q in sv[1]))
            nh = len(hvals)
            nhA = (nh + 1) // 2
            halves = [hvals[:nhA], hvals[nhA:]]
            hrank = {h: r for r, h in enumerate(hvals)}
            stash_t = {}
            src4s = {}
            for b in range(B_LOC):
                src4s[b] = feats[1][b].rearrange(
                    "(ch cp) h w -> cp ch h w", cp=128
                )

            def emit_rows(b, hf, engines):
                hs = halves[hf]
                st = l1pool.tile([128, 4 * len(hs) * H1], fr,
                                 tag=f"l1stash{hf}", name=f"l1st{b}{hf}")
                stash_t[(b, hf)] = st
                st4 = st[:].rearrange(
                    "c (ch hr w) -> c ch hr w", hr=len(hs), w=H1
                )
                for r, h in enumerate(hs):
                    eng = engines[0] if r < int(len(hs) * L1_ROW_SYNC_FRAC) \
                        else engines[1]
                    eng.dma_start(st4[:, :, r, :], src4s[b][:, :, h, :])

            emit_rows(0, 0, (nc.sync, nc.gpsimd))
            emit_rows(0, 1, (nc.sync, nc.gpsimd))

            # --- L0 gathers: contiguous sorted blocks per engine ---
            src3 = feats[0][:].rearrange(
                "b (ch cp) h w -> cp (b ch) (h w)", cp=128
            )
            dst3 = xts[0][:].rearrange("c (bc pp) -> c bc pp", pp=P)

            with nc.allow_non_contiguous_dma("sparse patch gather"):
                pos0 = 0
                for eng_name, n0 in GATHER_BLOCKS:
                    eng = getattr(nc, eng_name)
                    for i in range(pos0, pos0 + n0):
                        eng.dma_start(dst3[:, :, i], src3[:, :, int(sv[0][i])])
                    pos0 += n0
                assert pos0 == P

            # --- L1 extraction image 0: per-patch [128, 4] DVE copies ---
            xt1_4 = xts[1][:].rearrange(
                "c (b ch p) -> c b ch p", b=B_LOC, p=P
            )

            def emit_extract(b, split=False):
                st4s = [
                    stash_t[(b, hf)][:].rearrange(
                        "c (ch hr w) -> c ch hr w", hr=len(halves[hf]), w=H1
                    )
                    for hf in range(2)
                ]
                for i in range(P):
                    q = int(sv[1][i])
                    r = hrank[q >> 6]
                    hf = 0 if r < nhA else 1
                    eng = nc.gpsimd if (split and i % 2) else nc.vector
                    eng.tensor_copy(
                        xt1_4[:, b, :, i],
                        st4s[hf][:, :, r - hf * nhA, q & (H1 - 1)],
                    )

            emit_extract(0)

            # --- L2 full-compute: G=W1^T T (all q), H=relu(G), K=H^T W2,
            #     y = S^T K.  Both images' layer-1 back-to-back on PE. ---
            h2s = []
            gtags = ["gA", "gB", "gC", "gD"]
            for b in range(B_LOC):
                t2 = t2s[b]
                h2 = xtp.tile([128, 2 * HW2], fr, tag=f"h2_{b}", name=f"h2_{b}")
                for half in range(2):
                    gs = [
                        psum1.tile(
                            [128, 512], f32, tag=gtags[2 * b + qn],
                            name=gtags[2 * b + qn],
                        )
                        for qn in range(2)
                    ]
                    for cc in range(NCH2):
                        for qn in range(2):
                            nc.tensor.matmul(
                                gs[qn][:],
                                w1_sb[2][:, cc * NCD + half * 128 : cc * NCD + half * 128 + 128],
                                t2[:, cc * HW2 + qn * 512 : cc * HW2 + qn * 512 + 512],
                                start=(cc == 0),
                                stop=False,
                            )
                    for qn in range(2):
                        nc.tensor.matmul(  # + b1 broadcast over all q
                            gs[qn][:],
                            b1_sb[2][0:1, half * 128 : half * 128 + 128],
                            ones[0:1, 0:512],
                            start=False,
                            stop=True,
                        )
                        nc.scalar.activation(
                            h2[:, (half * 2 + qn) * 512 : (half * 2 + qn) * 512 + 512],
                            gs[qn][:],
                            AF.Relu,
                        )
                h2s.append(h2)

            # image-1 rows: A-half rides scalar (idle after the relus),
            # B-half rides sync after its L0 block
            emit_rows(1, 0, (nc.scalar, nc.scalar))
            emit_rows(1, 1, (nc.sync, nc.sync))

            # L0/L1 weights (cheap; needed once the gathers complete)
            load_weights(0, nc.scalar)
            load_weights(1, nc.scalar)

            for b in range(B_LOC):
                h2 = h2s[b]
                py = psum.tile([128, NCD], f32, tag="py", name="py2")
                for qc in range(QC2):
                    k = psum.tile([128, NCD], f32, tag="k", name="k")
                    for half in range(2):
                        o = (half * 2 + qc // 4) * 512 + (qc % 4) * 128
                        nc.tensor.matmul(
                            k[:],
                            h2[:, o : o + 128],
                            w2_sb[2][:, half * NCD : (half + 1) * NCD],
                            start=(half == 0),
                            stop=False,
                        )
                    nc.tensor.matmul(  # + b2 for every q (select sums to 1)
                        k[:],
                        ones[0:1, 0:128],
                        b2_sb[2][0:1, :],
                        start=False,
                        stop=True,
                    )
                    ksb = work.tile([128, NCD], fr, tag="ksb", name="ksb")
                    nc.vector.tensor_copy(ksb[:], k[:])
                    nc.tensor.matmul(
                        py[:],
                        oh_sb[:, qc * P : (qc + 1) * P],
                        ksb[:],
                        start=(qc == 0),
                        stop=(qc == QC2 - 1),
                    )
                _norm_and_store(nc, work, AF, f32, py, out, 2, b)

            emit_extract(1, split=True)

            # --- MLP for L0/L1 (both images batched into N=256) ---
            for l in (0, 1):
                C, H = LEVELS[l]
                n_ch = C // 128
                x4 = xts[l][:].rearrange("c (b ch p) -> c ch b p", b=B_LOC, p=P)
                hts = []
                for half in range(2):
                    ph = psum1.tile([128, 512], f32, tag=gtags[half], name="ph")
                    for ch in range(n_ch):
                        o = ch * NCD + half * 128
                        nc.tensor.matmul(
                            ph[:, 0 : B_LOC * P],
                            w1_sb[l][:, o : o + 128],
                            x4[:, ch],
                            start=(ch == 0),
                            stop=False,
                        )
                    nc.tensor.matmul(  # + b1 (rank-1)
                        ph[:, 0 : B_LOC * P],
                        b1_sb[l][0:1, half * 128 : half * 128 + 128],
                        ones[0:1, 0 : B_LOC * P],
                        start=False,
                        stop=True,
                    )
                    ht = work.tile([128, B_LOC * P], fr, tag="ht", name="ht")
                    nc.scalar.activation(ht[:], ph[:, 0 : B_LOC * P], AF.Relu)
                    hts.append(ht)

                for b in range(B_LOC):
                    py = psum.tile([128, NCD], f32, tag="py", name="py")
                    for half in range(2):
                        nc.tensor.matmul(
                            py[:],
                            hts[half][:, b * P : (b + 1) * P],
                            w2_sb[l][:, half * NCD : (half + 1) * NCD],
                            start=(half == 0),
                            stop=False,
                        )
                    nc.tensor.matmul(  # + b2 (rank-1)
                        py[:],
                        ones[0:1, 0:P],
                        b2_sb[l][0:1, :],
                        start=False,
                        stop=True,
                    )
                    _norm_and_store(nc, work, AF, f32, py, out, l, b)

    nc.compile()
    return nc


def _norm_and_store(nc, work, AF, f32, py, out, l, b):
    sq = work.tile([128, NCD], f32, tag="sq", name="sq")
    ssq = work.tile([128, 1], f32, tag="ssq", name="ssq")
    nc.scalar.activation(sq[:], py[:], AF.Square, accum_out=ssq[:])
    inv = work.tile([128, 1], f32, tag="inv", name="inv")
    nrm = work.tile([128, 1], f32, tag="nrm", name="nrm")
    nc.scalar.sqrt(nrm[:], ssq[:])
    nc.vector.reciprocal(inv[:], nrm[:])
    yo = work.tile([128, NCD], f32, tag="yo", name="yo")
    nc.scalar.mul(yo[:], py[:], inv[:])
    store_eng = [nc.sync, nc.gpsimd][(2 * l + b) % 2]
    store_eng.dma_start(out[l, b], yo[:])


def _run(inputs, trace=False):
    from concourse.bass_utils import run_bass_kernel_spmd

    feats = [
        np.ascontiguousarray(np.asarray(inputs[f"feat{l}"], dtype=np.float32))
        for l in range(3)
    ]
    idxs = [np.asarray(inputs[f"idx{l}"]).astype(np.int64) for l in range(3)]
    nc = _build(idxs)

    oh2 = np.zeros((8, 128, P), np.float32)
    for p, q in enumerate(idxs[2]):
        oh2[int(q) // 128, int(q) % 128, p] = 1.0

    in_maps = []
    for c in range(N_CORES):
        m = {"oh2": oh2}
        for l in range(3):
            m[f"feat{l}"] = feats[l][c * B_LOC : (c + 1) * B_LOC]
            m[f"w1_{l}"] = np.asarray(inputs[f"w1_{l}"], dtype=np.float32)
            m[f"b1_{l}"] = np.asarray(inputs[f"b1_{l}"], dtype=np.float32)
            m[f"w2_{l}"] = np.asarray(inputs[f"w2_{l}"], dtype=np.float32)
            m[f"b2_{l}"] = np.asarray(inputs[f"b2_{l}"], dtype=np.float32)
        in_maps.append(m)

    res = run_bass_kernel_spmd(
        nc, in_maps, core_ids=list(range(N_CORES)), trace=trace
    )
    full = np.concatenate([r["out"] for r in res.results], axis=1)
    # levels 0/1 were gathered in sorted-index order; un-permute patches
    for l in (0, 1):
        order = np.argsort(idxs[l], kind="stable")
        unperm = np.empty_like(full[l])
        unperm[:, order, :] = full[l]
        full[l] = unperm
    return full.astype(np.float32), res


def kernel(**inputs) -> np.ndarray:
    out, _ = _run(inputs, trace=False)
    return out



# revision 3
# speedup vs baseline: 1.1528x; 1.1528x over previous
"""NPMLPHead sampling kernel for Trainium2 (Bass/Tile), SPMD over 8 cores.

Strategy v2 (data-parallel over batch; merged-span gathers):
  - B=16 -> 2 images per core; full (tiny) MLP weights replicated per core.
  - The binding resource is DMA *descriptor* throughput: ~11ns fixed per
    descriptor per DMA engine (16 engines/core), ~25.6GB/s per engine for
    byte-heavy descriptors. A per-element patch gather costs 1 descriptor
    per 4B element (65K descriptors for L0 alone).
  - L0/L1: gather *merged spans*: sort the patch positions, merge
    neighbours closer than GAP_THR elements into one contiguous DRAM run,
    and DMA each run for all (image, channel) pairs in ONE instruction
    ([cp, b*ch, len] -> 128*b*ch descriptors of len*4B). This trades a few
    gap bytes (read and ignored) for a large cut in descriptor count.
    Patches are then extracted from the SBUF stash with one [128, b*ch]
    copy per (sorted) patch, split across DVE/Act/Pool by a weighted
    pattern. The host un-permutes the patch axis afterwards.
  - L2 (128 of 1K, C=1024): stream the whole shard with 4KB descriptors
    (cc-chunked, double-buffered), compute the MLP on ALL positions in the
    native [C, HW] layout (contraction on partitions), select patches at
    the end with a one-hot PE matmul (q lands on partitions via the
    layer-2 stationary operand).
  - Ring split: sync (HWDGE) carries most L0 runs; gpsimd (SWDGE, descgen
    on DSP0/1) carries L1 + the rest of L0; scalar (HWDGE) carries t2
    chunks + weights, keeping its compute chain (relu/norm) light.
  - All matmuls float32r (single-pass fp32, ~tf32 rounding); norm =
    Square+accum -> sqrt -> reciprocal -> mul (eps dropped; |y| >> eps).
"""

import sys

sys.path.insert(0, "/opt/trn_rl_repo")

import numpy as np

B = 16
N_CORES = 8
B_LOC = B // N_CORES  # 2
P = 128  # NUM_PATCHES
NCD = 256  # MLP width
LEVELS = [(256, 128), (512, 64), (1024, 32)]  # (C, H) per level
EPS = 1e-7

# Span-merge gap thresholds (elements) per gather level, and SBUF budget
# (bytes per partition) the stash must fit in (auto-shrink threshold if a
# pathological idx distribution would blow the budget).
GAP_THR = {0: 64, 1: 48}
STASH_BUDGET = {0: 45 * 1024, 1: 85 * 1024}
# Fraction of L0 run-instructions on the sync HWDGE ring (rest -> SWDGE).
L0_SYNC_FRAC = 0.65
# Extraction engine pattern (v=vector/DVE, s=scalar/Act, g=gpsimd/Pool).
EXTR_PATTERN = "vvvsvvvsvvvsvvgs"


def _span_runs(idx_sorted, thr):
    """Merge sorted positions into runs with gaps <= thr. Returns
    (runs=[(lo, ln, off)], cov, off_of: q -> concat offset)."""
    uq = np.unique(np.asarray(idx_sorted))
    bounds = []
    lo = hi = int(uq[0])
    for v in uq[1:]:
        v = int(v)
        if v - hi - 1 <= thr:
            hi = v
        else:
            bounds.append((lo, hi))
            lo = hi = v
    bounds.append((lo, hi))
    runs = []
    o = 0
    for lo_, hi_ in bounds:
        runs.append((lo_, hi_ - lo_ + 1, o))
        o += hi_ - lo_ + 1

    def off_of(q):
        for lo_, ln_, ob in runs:
            if lo_ <= q < lo_ + ln_:
                return ob + (q - lo_)
        raise ValueError(q)

    return runs, o, off_of


def _pick_runs(idx_sorted, thr, n_ch, budget):
    while True:
        runs, cov, off_of = _span_runs(idx_sorted, thr)
        if B_LOC * n_ch * cov * 4 <= budget or thr == 0:
            return runs, cov, off_of
        thr = thr // 2 if thr > 4 else 0


def _build(idx_vals):
    """Build the per-core Bass kernel. idx_vals: 3 int arrays of 128 patch
    ids. L0/L1 are gathered in sorted-index order (host un-permutes)."""
    import concourse.bass as bass
    import concourse.tile as tile
    from concourse import bacc, mybir

    f32 = mybir.dt.float32
    fr = mybir.dt.float32r
    AF = mybir.ActivationFunctionType

    nc = bacc.Bacc(None)

    feats, w1s, b1s, w2s, b2s = [], [], [], [], []
    for l, (C, H) in enumerate(LEVELS):
        feats.append(
            nc.dram_tensor(f"feat{l}", [B_LOC, C, H, H], fr, kind="ExternalInput")
        )
        w1s.append(nc.dram_tensor(f"w1_{l}", [C, NCD], fr, kind="ExternalInput"))
        b1s.append(nc.dram_tensor(f"b1_{l}", [NCD], fr, kind="ExternalInput"))
        w2s.append(nc.dram_tensor(f"w2_{l}", [NCD, NCD], fr, kind="ExternalInput"))
        b2s.append(nc.dram_tensor(f"b2_{l}", [NCD], fr, kind="ExternalInput"))
    # one-hot select for level 2 (original patch order)
    oh = nc.dram_tensor("oh2", [8, 128, P], fr, kind="ExternalInput")
    out = nc.dram_tensor("out", [3, B_LOC, P, NCD], f32, kind="ExternalOutput")

    C2, H2 = LEVELS[2]
    HW2 = H2 * H2  # 1024
    NCH2 = C2 // 128  # 8
    QC2 = HW2 // 128  # 8 q-chunks

    sv = {l: np.sort(np.asarray(idx_vals[l]).astype(np.int64)) for l in (0, 1)}
    runs, covs, offf = {}, {}, {}
    for l in (0, 1):
        n_ch = LEVELS[l][0] // 128
        runs[l], covs[l], offf[l] = _pick_runs(
            sv[l], GAP_THR[l], n_ch, STASH_BUDGET[l]
        )

    with tile.TileContext(nc) as tc:
        with (
            tc.tile_pool(name="consts", bufs=1) as consts,
            tc.tile_pool(name="stash", bufs=1) as stash,
            tc.tile_pool(name="t2p", bufs=3) as t2p,
            tc.tile_pool(name="xt", bufs=1) as xtp,
            tc.tile_pool(name="work", bufs=4) as work,
            tc.tile_pool(name="psum", bufs=2, space=bass.MemorySpace.PSUM) as psum,
            tc.tile_pool(name="psum1", bufs=1, space=bass.MemorySpace.PSUM) as psum1,
        ):
            ones_f = consts.tile([1, 512], f32, tag="ones_f")
            nc.vector.memset(ones_f[:], 1.0)
            ones = consts.tile([1, 512], fr, tag="ones")
            nc.scalar.copy(ones[:], ones_f[:])

            w1_sb, w2_sb, b1_sb, b2_sb = {}, {}, {}, {}

            def load_weights(l, eng):
                C, H = LEVELS[l]
                n_ch = C // 128
                t = consts.tile([128, n_ch * NCD], fr, tag=f"w1_{l}", name=f"w1sb{l}")
                eng.dma_start(
                    t[:].rearrange("cp (ch n) -> cp ch n", n=NCD),
                    w1s[l][:].rearrange("(ch cp) n -> cp ch n", cp=128),
                )
                w1_sb[l] = t
                t = consts.tile([128, 2 * NCD], fr, tag=f"w2_{l}", name=f"w2sb{l}")
                eng.dma_start(
                    t[:].rearrange("cp (ch n) -> cp ch n", n=NCD),
                    w2s[l][:].rearrange("(ch cp) n -> cp ch n", cp=128),
                )
                w2_sb[l] = t
                t = consts.tile([1, NCD], fr, tag=f"b1_{l}", name=f"b1sb{l}")
                eng.dma_start(t[:], b1s[l][:].rearrange("(o n) -> o n", o=1))
                b1_sb[l] = t
                t = consts.tile([1, NCD], fr, tag=f"b2_{l}", name=f"b2sb{l}")
                eng.dma_start(t[:], b2s[l][:].rearrange("(o n) -> o n", o=1))
                b2_sb[l] = t

            load_weights(2, nc.scalar)
            oh_sb = consts.tile([128, QC2 * P], fr, tag="oh2")
            nc.scalar.dma_start(
                oh_sb[:].rearrange("ql (qc p) -> ql qc p", p=P),
                oh[:].rearrange("qc ql p -> ql qc p"),
            )

            # --- L0/L1 merged-span gathers: one instr per run covering
            # both images and all channel chunks ---
            src_sp, stv, xts = {}, {}, {}
            for l in (0, 1):
                C, H = LEVELS[l]
                n_ch = C // 128
                src_sp[l] = feats[l][:].rearrange(
                    "b (ch cp) h w -> cp (b ch) (h w)", cp=128
                )
                t = stash.tile(
                    [128, B_LOC * n_ch * covs[l]], fr, tag=f"st{l}", name=f"st{l}"
                )
                stv[l] = t[:].rearrange("c (bc v) -> c bc v", v=covs[l])
                xts[l] = xtp.tile(
                    [128, B_LOC * n_ch * P], fr, tag=f"xt_{l}", name=f"xt{l}"
                )

            n_sync0 = int(len(runs[0]) * L0_SYNC_FRAC)
            with nc.allow_non_contiguous_dma("merged-span patch gather"):
                # interleave L0/L1 runs so both levels stream early
                em = []
                r1 = list(runs[1])
                step = max(1, len(runs[0]) // max(1, len(runs[1])))
                for i, r in enumerate(runs[0]):
                    eng = nc.sync if i < n_sync0 else nc.gpsimd
                    em.append((0, r, eng))
                    if i % step == step - 1 and r1:
                        em.append((1, r1.pop(0), nc.gpsimd))
                for r in r1:
                    em.append((1, r, nc.gpsimd))
                for l, (lo, ln, off), eng in em:
                    eng.dma_start(
                        stv[l][:, :, off : off + ln],
                        src_sp[l][:, :, lo : lo + ln],
                    )

            # --- L2 full-compute, t2 cc-chunk streamed (double-buffered):
            # G=W1^T T (all q), H=relu(G) ---
            src2 = [
                feats[2][b].rearrange("(ch cp) h w -> cp ch (h w)", cp=128)
                for b in range(B_LOC)
            ]
            h2s = []
            for b in range(B_LOC):
                gs = [
                    psum1.tile([128, 512], f32, tag=f"g{j}", name=f"g{j}_{b}")
                    for j in range(4)
                ]
                for cc in range(NCH2):
                    tc2t = t2p.tile([128, HW2], fr, tag="t2c", name=f"t2c{b}_{cc}")
                    nc.scalar.dma_start(tc2t[:], src2[b][:, cc, :])
                    for half in range(2):
                        for qn in range(2):
                            nc.tensor.matmul(
                                gs[half * 2 + qn][:],
                                w1_sb[2][
                                    :,
                                    cc * NCD + half * 128 : cc * NCD + half * 128 + 128,
                                ],
                                tc2t[:, qn * 512 : qn * 512 + 512],
                                start=(cc == 0),
                                stop=False,
                            )
                h2 = xtp.tile([128, 2 * HW2], fr, tag=f"h2_{b}", name=f"h2_{b}")
                for half in range(2):
                    for qn in range(2):
                        nc.tensor.matmul(  # + b1 broadcast over all q
                            gs[half * 2 + qn][:],
                            b1_sb[2][0:1, half * 128 : half * 128 + 128],
                            ones[0:1, 0:512],
                            start=False,
                            stop=True,
                        )
                        nc.scalar.activation(
                            h2[:, (half * 2 + qn) * 512 : (half * 2 + qn) * 512 + 512],
                            gs[half * 2 + qn][:],
                            AF.Relu,
                        )
                h2s.append(h2)

            # L0/L1 weights (cheap; needed for the final MLPs)
            load_weights(0, nc.scalar)
            load_weights(1, nc.scalar)

            # --- extraction: one [128, b*ch] copy per sorted patch,
            # engines by weighted pattern; L0/L1 interleaved ---
            xtv = {
                l: xts[l][:].rearrange(
                    "c (b ch p) -> c b ch p", b=B_LOC, p=P
                )
                for l in (0, 1)
            }
            stv4 = {
                l: stv[l].rearrange(
                    "c (b ch) v -> c b ch v", b=B_LOC
                )
                for l in (0, 1)
            }
            engs = {"v": nc.vector, "s": nc.scalar, "g": nc.gpsimd}
            ei = 0
            for i in range(P):
                for l in (0, 1):
                    o = offf[l](int(sv[l][i]))
                    eng = engs[EXTR_PATTERN[ei % len(EXTR_PATTERN)]]
                    ei += 1
                    if eng is nc.scalar:
                        eng.copy(xtv[l][:, :, :, i], stv4[l][:, :, :, o])
                    else:
                        eng.tensor_copy(xtv[l][:, :, :, i], stv4[l][:, :, :, o])

            # --- L2 layer 2 + one-hot select + norm/store ---
            for b in range(B_LOC):
                h2 = h2s[b]
                py = psum.tile([128, NCD], f32, tag="py", name="py2")
                for qc in range(QC2):
                    k = psum.tile([128, NCD], f32, tag="k", name="k")
                    for half in range(2):
                        o = (half * 2 + qc // 4) * 512 + (qc % 4) * 128
                        nc.tensor.matmul(
                            k[:],
                            h2[:, o : o + 128],
                            w2_sb[2][:, half * NCD : (half + 1) * NCD],
                            start=(half == 0),
                            stop=False,
                        )
                    nc.tensor.matmul(  # + b2 for every q (select sums to 1)
                        k[:],
                        ones[0:1, 0:128],
                        b2_sb[2][0:1, :],
                        start=False,
                        stop=True,
                    )
                    ksb = work.tile([128, NCD], fr, tag="ksb", name="ksb")
                    nc.vector.tensor_copy(ksb[:], k[:])
                    nc.tensor.matmul(
                        py[:],
                        oh_sb[:, qc * P : (qc + 1) * P],
                        ksb[:],
                        start=(qc == 0),
                        stop=(qc == QC2 - 1),
                    )
                _norm_and_store(nc, work, AF, f32, py, out, 2, b)

            # --- MLP for L0/L1 (both images batched into N=256) ---
            for l in (0, 1):
                C, H = LEVELS[l]
                n_ch = C // 128
                x4 = xts[l][:].rearrange("c (b ch p) -> c ch b p", b=B_LOC, p=P)
                hts = []
                for half in range(2):
                    ph = psum1.tile([128, 512], f32, tag=f"g{half}", name="ph")
                    for ch in range(n_ch):
                        o = ch * NCD + half * 128
                        nc.tensor.matmul(
                            ph[:, 0 : B_LOC * P],
                            w1_sb[l][:, o : o + 128],
                            x4[:, ch],
                            start=(ch == 0),
                            stop=False,
                        )
                    nc.tensor.matmul(  # + b1 (rank-1)
                        ph[:, 0 : B_LOC * P],
                        b1_sb[l][0:1, half * 128 : half * 128 + 128],
                        ones[0:1, 0 : B_LOC * P],
                        start=False,
                        stop=True,
                    )
                    ht = work.tile([128, B_LOC * P], fr, tag="ht", name="ht")
                    nc.scalar.activation(ht[:], ph[:, 0 : B_LOC * P], AF.Relu)
                    hts.append(ht)

                for b in range(B_LOC):
                    py = psum.tile([128, NCD], f32, tag="py", name="py")
                    for half in range(2):
                        nc.tensor.matmul(
                            py[:],
                            hts[half][:, b * P : (b + 1) * P],
                            w2_sb[l][:, half * NCD : (half + 1) * NCD],
                            start=(half == 0),
                            stop=False,
                        )
                    nc.tensor.matmul(  # + b2 (rank-1)
                        py[:],
                        ones[0:1, 0:P],
                        b2_sb[l][0:1, :],
                        start=False,
                        stop=True,
                    )
                    _norm_and_store(nc, work, AF, f32, py, out, l, b)

    nc.compile()
    return nc


def _norm_and_store(nc, work, AF, f32, py, out, l, b):
    sq = work.tile([128, NCD], f32, tag="sq", name="sq")
    ssq = work.tile([128, 1], f32, tag="ssq", name="ssq")
    nc.scalar.activation(sq[:], py[:], AF.Square, accum_out=ssq[:])
    inv = work.tile([128, 1], f32, tag="inv", name="inv")
    nrm = work.tile([128, 1], f32, tag="nrm", name="nrm")
    nc.scalar.sqrt(nrm[:], ssq[:])
    nc.vector.reciprocal(inv[:], nrm[:])
    yo = work.tile([128, NCD], f32, tag="yo", name="yo")
    nc.scalar.mul(yo[:], py[:], inv[:])
    store_eng = [nc.sync, nc.gpsimd][(2 * l + b) % 2]
    store_eng.dma_start(out[l, b], yo[:])


def _run(inputs, trace=False):
    from concourse.bass_utils import run_bass_kernel_spmd

    feats = [
        np.ascontiguousarray(np.asarray(inputs[f"feat{l}"], dtype=np.float32))
        for l in range(3)
    ]
    idxs = [np.asarray(inputs[f"idx{l}"]).astype(np.int64) for l in range(3)]
    nc = _build(idxs)

    oh2 = np.zeros((8, 128, P), np.float32)
    for p, q in enumerate(idxs[2]):
        oh2[int(q) // 128, int(q) % 128, p] = 1.0

    in_maps = []
    for c in range(N_CORES):
        m = {"oh2": oh2}
        for l in range(3):
            m[f"feat{l}"] = feats[l][c * B_LOC : (c + 1) * B_LOC]
            m[f"w1_{l}"] = np.asarray(inputs[f"w1_{l}"], dtype=np.float32)
            m[f"b1_{l}"] = np.asarray(inputs[f"b1_{l}"], dtype=np.float32)
            m[f"w2_{l}"] = np.asarray(inputs[f"w2_{l}"], dtype=np.float32)
            m[f"b2_{l}"] = np.asarray(inputs[f"b2_{l}"], dtype=np.float32)
        in_maps.append(m)

    res = run_bass_kernel_spmd(
        nc, in_maps, core_ids=list(range(N_CORES)), trace=trace
    )
    full = np.concatenate([r["out"] for r in res.results], axis=1)
    # levels 0/1 were gathered in sorted-index order; un-permute patches
    for l in (0, 1):
        order = np.argsort(idxs[l], kind="stable")
        unperm = np.empty_like(full[l])
        unperm[:, order, :] = full[l]
        full[l] = unperm
    return full.astype(np.float32), res


def kernel(**inputs) -> np.ndarray:
    out, _ = _run(inputs, trace=False)
    return out
